# revision 54
# baseline (speedup 1.0000x reference)
"""Causal attention head (B=4, S=4096, D=512, E=64) on 8 TRN2 NeuronCores.

Sharding: per batch b, core pair (2b, 2b+1); zig-zag query-block pairs.
 - Each core owns 4 query blocks of 512 as two pairs (g, 7-g):
   parity 0: (3,4) then (0,7);  parity 1: (2,5) then (1,6).
   Every pair-slot covers exactly 16896 score rows -> perfectly balanced.
 - Both cores project the FULL K/V sequence locally (no collectives).
 - Flash-style attention on transposed scores S^T = K^T_chunk^T @ Q^T with
   shared-chunk matmuls spanning both blocks of a pair (<=512-col segments,
   PSUM-bank aligned) and causal windows refined to 128-col granularity.
 - exp on ScalarE only; PSUM->SBUF copies on DVE, triangle masks on GpSimd.
   The two score matmuls of every unit run concurrently on split PE tiles
   (rows 0-63 / 64-127, K^T/Q^T dual-copied), writing different PSUM banks.
   PV accumulates with a ones-column appended to V so the softmax
   denominator falls out of the same matmul.
 - V is projected as V^T with big-N matmuls (32 instrs instead of 128 tiny
   ones) and PE-transposed back in 4-chunk batches.
 - Zero-matmul warmup ramps the PE pstate while the first DMA pieces land.
 - Z^T (incl. denominator row) is DMA'd out raw; the divide + transpose
   happen on host.  Each PSUM bank's half is shipped as soon as its last
   PV lands.
 - Per-parity schedules selected with tc.If on partition_id()%2.
 - Input DMA striped across three hardware queues (sync/scalar/gpsimd) in
   compute-priority order; projections are emitted lazily right before the
   attention units that consume them.
All matmul inputs bf16 (pre-cast/transposed on host); output f32.
"""

import sys

sys.path.insert(0, "/opt/trn_rl_repo")

import numpy as np
import ml_dtypes

from concourse import bacc, mybir
from concourse import tile
from concourse.bass_utils import run_bass_kernel_spmd

BF16 = ml_dtypes.bfloat16
F32 = mybir.dt.float32
BF = mybir.dt.bfloat16

B, S, D, E = 4, 4096, 512, 64
P = 128
NQ = 2048          # queries per core
NCH = D // P       # 4 contraction chunks
NKCH = S // P      # 32 key chunks
PAIR_G = {0: (3, 0), 1: (2, 1)}  # parity -> (gA, gB); pair-slot = blocks (g, 7-g)

# projection group column ranges (narrow up front so compute starts early)
KGROUPS = [(0, 256), (256, 512)] + [(512 * i, 512 * (i + 1)) for i in range(1, 8)]
QGROUPS = [(0, 256), (256, 512)] + [(512 * i, 512 * (i + 1)) for i in range(1, 4)]

_CACHE = {}
LAST_RESULT = None


def _block_order(h):
    gA, gB = PAIR_G[h]
    return [gA, 7 - gA, gB, 7 - gB]


def _chunk_windows(g):
    """Chunk schedule for pair-slot (g, 7-g): list of (j, w0, tri).

    Window = pair-local query cols [w0, 1024); tri marks chunks whose first
    128 window cols need the causal triangle mask."""
    nch = 32 - 4 * g
    out = []
    for j in range(nch):
        if j < 4 * g:
            w0, tri = 0, False                       # full for both blocks
        elif j < 4 * g + 4:
            w0, tri = 128 * (j - 4 * g), True        # diag of block g
        elif j < 28 - 4 * g:
            w0, tri = 512, False                     # solo for block 7-g
        else:
            w0, tri = 512 + 128 * (j - (28 - 4 * g)), True  # diag of 7-g
        out.append((j, w0, tri))
    return out


def _pack_tiles(chunks):
    """Greedy-pack chunk windows into [128,1024] score tiles (one exp each).

    Matmul outputs must not cross the PSUM bank boundary (512 f32 cols), so
    a chunk with w0 < 512 (window wider than 512) sits alone at its natural
    position [w0, 1024).  Narrow (w0 >= 512) chunks are greedy-packed,
    flushing when a chunk would straddle the 512 boundary.  Tiles are
    (chunks, lo, hi): exp runs on [lo, hi); chunk entry (j, w0, tri, loc, n).
    """
    def flush(cur):
        hi = max(loc + n for (_j, _w, _t, loc, n) in cur)
        lo = min(loc for (_j, _w, _t, loc, n) in cur)
        return (cur, lo, hi)

    tiles, cur = [], []
    for (j, w0, tri) in chunks:
        n = 1024 - w0
        if w0 < 512:
            if cur:
                tiles.append(flush(cur))
                cur = []
            tiles.append(([(j, w0, tri, w0, n)], w0, 1024))
            continue
        # narrow chunks: two per tile, one per PSUM bank (loc 0 / 512) so
        # the paired score matmuls never write the same bank concurrently
        cur.append((j, w0, tri, 512 * len(cur), n))
        if len(cur) == 2:
            tiles.append(flush(cur))
            cur = []
    if cur:
        tiles.append(flush(cur))
    return tiles


def _weave(plain, diag, k=2):
    """Interleave diag (PE-light, ACT-heavy) units among plain units."""
    out, pi, di = [], 0, 0
    while pi < len(plain) or di < len(diag):
        for _ in range(k):
            if pi < len(plain):
                out.append(plain[pi]); pi += 1
        if di < len(diag):
            out.append(diag[di]); di += 1
    return out


def _unit_order(g):
    tiles = _pack_tiles(_chunk_windows(g))
    plain = [t for t in tiles if not any(c[2] for c in t[0])]
    diag = [t for t in tiles if any(c[2] for c in t[0])]
    return _weave(plain, diag)


def _segs(w0):
    # matmul free dim is capped at 512 f32 (one PSUM bank), bank-aligned
    return [(w0, 512), (512, 1024)] if w0 < 512 else [(w0, 1024)]


def _build():
    nc = bacc.Bacc(
        "TRN2",
        target_bir_lowering=False,
        debug=False,
        enable_asserts=True,
        num_devices=8,
    )

    xqt_d = nc.declare_dram_parameter("xqt", [P, NCH * NQ], BF, isOutput=False)
    xkt_d = nc.declare_dram_parameter("xkt", [P, NCH * S], BF, isOutput=False)
    xvt_d = nc.declare_dram_parameter("xvt", [P, NCH * S], BF, isOutput=False)
    wq_d = nc.declare_dram_parameter("wq", [P, NCH * E], BF, isOutput=False)
    wk_d = nc.declare_dram_parameter("wk", [P, NCH * E], BF, isOutput=False)
    wv_d = nc.declare_dram_parameter("wv", [P, NCH * E], BF, isOutput=False)
    tri_d = nc.declare_dram_parameter("tri", [P, P], BF, isOutput=False)
    ident_d = nc.declare_dram_parameter("ident", [E, E], BF, isOutput=False)
    zout = nc.declare_dram_parameter("z", [2 * (E + 1), 1024], F32, isOutput=True)

    with tile.TileContext(nc) as tc:
        with (
            tc.tile_pool(name="const", bufs=1) as const,
            tc.tile_pool(name="xt", bufs=1) as xt,
            tc.tile_pool(name="proj", bufs=1) as proj,
            tc.tile_pool(name="work", bufs=3) as work,
            tc.tile_pool(name="epi", bufs=2) as epi,
            tc.tile_pool(name="psA", bufs=2, space="PSUM") as psA,
            tc.tile_pool(name="psZ", bufs=1, space="PSUM") as psZ,
            tc.tile_pool(name="psP", bufs=2, space="PSUM") as psP,
        ):
            wq_sb = const.tile([P, NCH, E], BF, tag="wq")
            wk_sb = const.tile([P, NCH, E], BF, tag="wk")
            wv_sb = const.tile([P, NCH, E], BF, tag="wv")
            tri_sb = const.tile([P, P], BF, tag="tri")
            ident_sb = const.tile([E, E], BF, tag="ident")

            xqt = xt.tile([P, NCH * NQ], BF, tag="xqt")
            xkt = xt.tile([P, NCH * S], BF, tag="xkt")
            xvt = xt.tile([P, NCH * S], BF, tag="xvt")

            kt = proj.tile([P, S], BF, tag="kt")        # K^T, duplicated in
            # partitions 0-63 and 64-127 so score matmuls can run pairwise on
            # split PE tiles (rows 0-63 / 64-127) and stream concurrently.
            qt = proj.tile([P, NQ], BF, tag="qt")       # Q^T, dual copy like kt
            vt = proj.tile([E, S], BF, tag="vt")        # V^T (pre-transpose)
            vp = proj.tile([P, NKCH, E + 1], BF, tag="vp")
            nc.gpsimd.memset(vp[:, :, E : E + 1], 1.0)

            # ---------------- DMA plan: 3 queues, priority order -----------
            SQ, AQ, PQ = nc.sync, nc.scalar, nc.gpsimd

            def wdma(qeng, dst, src):
                qeng.dma_start(
                    out=dst[:, :, :], in_=src.rearrange("p (c e) -> p c e", c=NCH)
                )

            def xpiece(qeng, dst, src, a, b):
                qeng.dma_start(
                    out=dst[:, :].rearrange("p (c s) -> p c s", c=NCH)[:, :, a:b],
                    in_=src.rearrange("p (c s) -> p c s", c=NCH)[:, :, a:b],
                )

            # Front: Q split over sync+scalar (it gates attention start), K/V
            # first pieces on gpsimd; bulk striped over all three queues.
            # Scalar (ACT) carries few issues so its queue drains before exp.
            wdma(AQ, wq_sb, wq_d)
            wdma(SQ, wk_sb, wk_d)
            wdma(PQ, wv_sb, wv_d)
            xpiece(AQ, xqt, xqt_d, 0, 256)
            xpiece(SQ, xqt, xqt_d, 256, 512)
            xpiece(PQ, xkt, xkt_d, 0, 256)
            xpiece(AQ, xqt, xqt_d, 512, 768)
            xpiece(SQ, xqt, xqt_d, 768, 1024)
            xpiece(PQ, xvt, xvt_d, 0, 256)
            AQ.dma_start(out=tri_sb[:, :], in_=tri_d[:, :])
            AQ.dma_start(out=ident_sb[:, :], in_=ident_d[:, :])

            # partition id loads before the bulk queue traffic
            pid = nc.partition_id()

            # PE pstate warmup: the tensor engine ramps to full clock only
            # after ~3us of continuous work.  While the first DMA pieces are
            # in flight the PE is idle anyway — feed it zero matmuls so the
            # real projections start at full speed.
            warm = const.tile([E, 512], BF, tag="warm")
            nc.vector.memset(warm[:, :], 0.0)
            for _ in range(16):
                wps = psP.tile([P, 512], F32, tag="pp")
                nc.tensor.matmul(
                    wps,
                    lhsT=warm[:, 0:P],
                    rhs=warm[:, :],
                    start=True,
                    stop=True,
                )

            # bulk on sync/gpsimd only: a DMA issue costs ~1.1us of the
            # issuing engine's time, and scalar (ACT) must spend its budget
            # on the exp stream instead
            for (t, a, b, q) in [
                ("k", 256, 512, SQ), ("v", 256, 512, PQ),
                ("k", 512, 1024, SQ), ("v", 512, 1024, PQ),
                ("k", 1024, 1536, SQ), ("v", 1024, 1536, PQ),
                ("k", 1536, 2048, PQ), ("v", 1536, 2048, SQ),
                ("q", 1024, 1536, SQ), ("q", 1536, 2048, PQ),
                ("k", 2048, 2560, SQ), ("v", 2048, 2560, PQ),
                ("k", 2560, 3072, SQ), ("v", 2560, 3072, PQ),
                ("k", 3072, 3584, SQ), ("v", 3072, 3584, PQ),
                ("k", 3584, 4096, SQ), ("v", 3584, 4096, PQ),
            ]:
                dst, src = {"k": (xkt, xkt_d), "v": (xvt, xvt_d), "q": (xqt, xqt_d)}[t]
                xpiece(q, dst, src, a, b)

            # ---------------- projection emitters (lazy) --------------------
            def emit_kgroup(a, b):
                ps = psP.tile([E, b - a], F32, tag="pp")
                for c in range(NCH):
                    nc.tensor.matmul(
                        ps,
                        lhsT=wk_sb[:, c, :],
                        rhs=xkt[:, c * S + a : c * S + b],
                        start=(c == 0),
                        stop=(c == NCH - 1),
                    )
                nc.vector.tensor_copy(kt[0:E, a:b], ps)
                nc.vector.tensor_copy(kt[E : 2 * E, a:b], ps)

            def emit_qgroup(a, b):
                ps = psP.tile([E, b - a], F32, tag="pp")
                for c in range(NCH):
                    nc.tensor.matmul(
                        ps,
                        lhsT=wq_sb[:, c, :],
                        rhs=xqt[:, c * NQ + a : c * NQ + b],
                        start=(c == 0),
                        stop=(c == NCH - 1),
                    )
                nc.vector.tensor_copy(qt[0:E, a:b], ps)
                nc.vector.tensor_copy(qt[E : 2 * E, a:b], ps)

            def emit_vtgroup(a, b):
                # V^T like K^T: big-N matmuls (32 instrs total instead of 128
                # tiny N=64 ones whose ldweights dominate PE time).
                ps = psP.tile([E, b - a], F32, tag="pp")
                for c in range(NCH):
                    nc.tensor.matmul(
                        ps,
                        lhsT=wv_sb[:, c, :],
                        rhs=xvt[:, c * S + a : c * S + b],
                        start=(c == 0),
                        stop=(c == NCH - 1),
                    )
                nc.vector.tensor_copy(vt[:, a:b], ps)

            state = {"kg": 0, "vg": 0, "vc": 0, "qg": 0}

            def ensure_k(upto_col):
                while state["kg"] < len(KGROUPS) and KGROUPS[state["kg"]][0] < upto_col:
                    a, b = KGROUPS[state["kg"]]
                    emit_kgroup(a, b)
                    state["kg"] += 1

            def ensure_v(upto_chunk):
                upto_chunk = min(upto_chunk, NKCH)
                while (
                    state["vg"] < len(KGROUPS)
                    and KGROUPS[state["vg"]][0] < P * upto_chunk
                ):
                    a, b = KGROUPS[state["vg"]]
                    emit_vtgroup(a, b)
                    state["vg"] += 1
                while state["vc"] < upto_chunk:
                    # transpose four 128-key chunks into one PSUM tile, then
                    # one DVE copy into vp (keeps the psP ring turnover low)
                    i0 = state["vc"]
                    nq = min(4, upto_chunk - i0)
                    tp = psP.tile([P, 4, E], BF, tag="pp")
                    for k in range(nq):
                        nc.tensor.transpose(
                            tp[:, k, :],
                            vt[:, P * (i0 + k) : P * (i0 + k + 1)],
                            ident_sb[:, :],
                        )
                    nc.vector.tensor_copy(vp[:, i0 : i0 + nq, 0:E], tp[:, 0:nq, :])
                    state["vc"] += nq

            def ensure_q(upto_col):
                while state["qg"] < len(QGROUPS) and QGROUPS[state["qg"]][0] < upto_col:
                    a, b = QGROUPS[state["qg"]]
                    emit_qgroup(a, b)
                    state["qg"] += 1

            # ---------------- attention slot emitter ------------------------
            def emit_slot(g, qbase, slot_idx):
                ensure_q(qbase + 1024)
                units = _unit_order(g)
                nch = 32 - 4 * g
                jlast_b0 = 4 * g + 3  # last chunk writing PSUM bank 0
                zp = psZ.tile([E + 1, 1024], F32, tag="zps")

                # execution order of chunks -> per-bank first/last writers
                order = [c for (tc_, lo, hi) in units for c in tc_]
                first_w = {0: None, 1: None}
                last_w = {0: None, 1: None}
                for (j, w0, tri, loc, n) in order:
                    for (a, b) in _segs(w0):
                        bank = a // 512
                        if first_w[bank] is None:
                            first_w[bank] = j
                        last_w[bank] = j
                last_b0_unit = max(
                    ui
                    for ui, (tc_, lo, hi) in enumerate(units)
                    if any(c[1] < 512 for c in tc_)
                )

                def ep_half(bank):
                    zs = epi.tile([E + 1, 512], F32, tag="zsb")
                    nc.vector.tensor_copy(zs, zp[:, 512 * bank : 512 * (bank + 1)])
                    (SQ if bank == 0 else AQ).dma_start(
                        out=zout[
                            (E + 1) * slot_idx : (E + 1) * (slot_idx + 1),
                            512 * bank : 512 * (bank + 1),
                        ],
                        in_=zs,
                    )

                def emit_pv(ui, tile_chunks, pt):
                    for (j, w0, _tri, loc, n) in tile_chunks:
                        for (a, b) in _segs(w0):
                            bank = a // 512
                            nc.tensor.matmul(
                                zp[:, a:b],
                                lhsT=vp[:, j, :],
                                rhs=pt[:, loc + (a - w0) : loc + (b - w0)],
                                start=(first_w[bank] == j),
                                stop=(last_w[bank] == j),
                                skip_group_check=True,
                            )
                    if ui == last_b0_unit:
                        ep_half(0)

                pending = []
                for ui, (tile_chunks, lo, hi) in enumerate(units):
                    jmax = max(c[0] for c in tile_chunks)
                    ensure_k(P * (jmax + 1))
                    ensure_v(jmax + 1)
                    sp = psA.tile([P, 1024], F32, tag="sps")
                    mms = [
                        (j, w0, a, b)
                        for (j, w0, _tri, loc, n) in tile_chunks
                        for (a, b) in _segs(w0)
                    ]
                    locs = {j: loc for (j, _w, _t, loc, _n) in tile_chunks}
                    assert len(mms) <= 2, mms
                    for h, (j, w0, a, b) in enumerate(mms):
                        loc = locs[j]
                        nc.tensor.matmul(
                            sp[:, loc + (a - w0) : loc + (b - w0)],
                            lhsT=kt[h * E : (h + 1) * E, P * j : P * (j + 1)],
                            rhs=qt[h * E : (h + 1) * E, qbase + a : qbase + b],
                            start=True,
                            stop=True,
                            tile_position=(h * E, 0),
                        )
                    pt = work.tile([P, 1024], BF, tag="pt")
                    nc.scalar.activation(
                        out=pt[:, lo:hi],
                        in_=sp[:, lo:hi],
                        func=mybir.ActivationFunctionType.Exp,
                    )
                    for (j, w0, tri, loc, n) in tile_chunks:
                        if tri:
                            nc.gpsimd.tensor_mul(
                                pt[:, loc : loc + P],
                                pt[:, loc : loc + P],
                                tri_sb[:, 0:P],
                            )
                    pending.append((ui, tile_chunks, pt))
                    if len(pending) > 2:
                        emit_pv(*pending.pop(0))
                for args in pending:
                    emit_pv(*args)
                ep_half(1)

            # ---------------- per-parity branches ---------------------------
            # Lazy-projection state must reset per branch: each tc.If body
            # needs its own copy of the post-preamble projections, since a
            # core executes only one branch.
            state_preamble = dict(state)

            def emit_branch(h):
                state.clear()
                state.update(state_preamble)
                gA, gB = PAIR_G[h]
                emit_slot(gA, 0, 0)
                ensure_q(NQ)
                ensure_k(S)
                ensure_v(NKCH)
                emit_slot(gB, 1024, 1)

            with tc.If(pid % 2 == 0):
                emit_branch(0)
            with tc.If(pid % 2 == 1):
                emit_branch(1)

    nc.compile()
    return nc


def _get_nc():
    if "nc" not in _CACHE:
        _CACHE["nc"] = _build()
    return _CACHE["nc"]


def _ensure_ntff_hook():
    """Install antenv.axon_hooks + NTFF profile hook if the image lacks it."""
    import types

    try:
        from antenv import axon_hooks  # noqa: F401

        return
    except ImportError:
        pass
    import antenv
    from concourse import bass_utils as _bu

    mod = types.ModuleType("antenv.axon_hooks")
    _state = {}
    mod.set_axon_ntff_profile_hook = lambda h: _state.__setitem__("h", h)
    mod.get_axon_ntff_profile_hook = lambda: _state.get("h")
    sys.modules["antenv.axon_hooks"] = mod
    antenv.axon_hooks = mod
    sys.path.insert(0, "/root/.axon_site/trn_agent_boot")
    from trn_boot import _ntff_profile_via_ctypes

    mod.set_axon_ntff_profile_hook(
        _ntff_profile_via_ctypes("/opt/axon/libaxon_pjrt.so")
    )
    _bu.upload_artifacts = lambda tmpdir: f"local://{tmpdir}"


def _to_sb_layout(xT):
    """[D, N] (f32) -> [128, NCH*N] bf16 in [p, (c n)] SBUF layout."""
    Dd, N = xT.shape
    return np.ascontiguousarray(
        xT.reshape(NCH, P, N).transpose(1, 0, 2).reshape(P, NCH * N)
    ).astype(BF16)


def kernel(key_inputs, value_inputs, query_inputs, Wq, Wk, Wv):
    global LAST_RESULT
    import os

    key_inputs = np.asarray(key_inputs, dtype=np.float32)
    value_inputs = np.asarray(value_inputs, dtype=np.float32)
    query_inputs = np.asarray(query_inputs, dtype=np.float32)
    wq_b = np.ascontiguousarray(
        (np.asarray(Wq, np.float32) * 0.125).reshape(NCH, P, E).transpose(1, 0, 2)
    ).astype(BF16).reshape(P, NCH * E)
    wk_b = np.ascontiguousarray(
        np.asarray(Wk, np.float32).reshape(NCH, P, E).transpose(1, 0, 2)
    ).astype(BF16).reshape(P, NCH * E)
    wv_b = np.ascontiguousarray(
        np.asarray(Wv, np.float32).reshape(NCH, P, E).transpose(1, 0, 2)
    ).astype(BF16).reshape(P, NCH * E)
    tri_np = (
        (np.arange(P)[:, None] <= np.arange(P)[None, :]).astype(np.float32)
    ).astype(BF16)
    ident_np = np.eye(E, dtype=np.float32).astype(BF16)

    in_maps = []
    for c in range(8):
        b, h = c // 2, c % 2
        order = _block_order(h)
        xq_loc = np.concatenate(
            [query_inputs[b, 512 * g : 512 * (g + 1)] for g in order], axis=0
        )
        in_maps.append(
            {
                "xqt": _to_sb_layout(xq_loc.T),
                "xkt": _to_sb_layout(key_inputs[b].T),
                "xvt": _to_sb_layout(value_inputs[b].T),
                "wq": wq_b,
                "wk": wk_b,
                "wv": wv_b,
                "tri": tri_np,
                "ident": ident_np,
            }
        )

    nc = _get_nc()
    trace = bool(int(os.environ.get("KERNEL_TRACE", "0")))
    if trace:
        _ensure_ntff_hook()
    res = run_bass_kernel_spmd(
        nc,
        in_maps,
        core_ids=list(range(8)),
        trace=trace,
        tmpdir=os.environ.get("KERNEL_TRACE_DIR") or None,
    )
    LAST_RESULT = res

    out = np.empty((B, S, E), dtype=np.float32)
    for c in range(8):
        b, h = c // 2, c % 2
        order = _block_order(h)
        zr = np.asarray(res.results[c]["z"], dtype=np.float32).reshape(2, E + 1, 1024)
        for s in range(2):
            zT = zr[s, 0:E, :] / zr[s, E, :][None, :]
            out[b, 512 * order[2 * s] : 512 * (order[2 * s] + 1)] = zT[:, 0:512].T
            out[b, 512 * order[2 * s + 1] : 512 * (order[2 * s + 1] + 1)] = zT[
                :, 512:1024
            ].T
    return out


# revision 55
# speedup vs baseline: 1.0251x; 1.0251x over previous
"""Causal attention head (B=4, S=4096, D=512, E=64) on 8 TRN2 NeuronCores.

Sharding: per batch b, core pair (2b, 2b+1); zig-zag query-block pairs.
 - Each core owns 4 query blocks of 512 as two pairs (g, 7-g):
   parity 0: (3,4) then (0,7);  parity 1: (2,5) then (1,6).
   Every pair-slot covers exactly 16896 score rows -> perfectly balanced.
 - Both cores project the FULL K/V sequence locally (no collectives).
 - Flash-style attention on transposed scores S^T = K^T_chunk^T @ Q^T with
   shared-chunk matmuls spanning both blocks of a pair (<=512-col segments,
   PSUM-bank aligned) and causal windows refined to 128-col granularity.
 - exp on ScalarE only; PSUM->SBUF copies on DVE, triangle masks on GpSimd.
   The two score matmuls of every unit run concurrently on split PE tiles
   (rows 0-63 / 64-127, K^T/Q^T dual-copied), writing different PSUM banks.
   PV accumulates with a ones-column appended to V so the softmax
   denominator falls out of the same matmul.
 - V is projected as V^T with big-N matmuls (32 instrs instead of 128 tiny
   ones) and PE-transposed back in 4-chunk batches.
 - Zero-matmul warmup ramps the PE pstate while the first DMA pieces land.
 - Z^T (incl. denominator row) is DMA'd out raw; the divide + transpose
   happen on host.  Each PSUM bank's half is shipped as soon as its last
   PV lands.
 - Per-parity schedules selected with tc.If on partition_id()%2.
 - Input DMA striped across three hardware queues (sync/scalar/gpsimd) in
   compute-priority order; projections are emitted lazily right before the
   attention units that consume them.
All matmul inputs bf16 (pre-cast/transposed on host); output f32.
"""

import sys

sys.path.insert(0, "/opt/trn_rl_repo")

import numpy as np
import ml_dtypes

from concourse import bacc, mybir
from concourse import tile
from concourse.bass_utils import run_bass_kernel_spmd

BF16 = ml_dtypes.bfloat16
F32 = mybir.dt.float32
BF = mybir.dt.bfloat16

B, S, D, E = 4, 4096, 512, 64
P = 128
NQ = 2048          # queries per core
NCH = D // P       # 4 contraction chunks
NKCH = S // P      # 32 key chunks
PAIR_G = {0: (3, 0), 1: (2, 1)}  # parity -> (gA, gB); pair-slot = blocks (g, 7-g)

# projection group column ranges (narrow up front so compute starts early)
KGROUPS = [(0, 256), (256, 512)] + [(512 * i, 512 * (i + 1)) for i in range(1, 8)]
QGROUPS = [(0, 256), (256, 512)] + [(512 * i, 512 * (i + 1)) for i in range(1, 4)]

_CACHE = {}
LAST_RESULT = None


def _block_order(h):
    gA, gB = PAIR_G[h]
    return [gA, 7 - gA, gB, 7 - gB]


def _chunk_windows(g):
    """Chunk schedule for pair-slot (g, 7-g): list of (j, w0, tri).

    Window = pair-local query cols [w0, 1024); tri marks chunks whose first
    128 window cols need the causal triangle mask."""
    nch = 32 - 4 * g
    out = []
    for j in range(nch):
        if j < 4 * g:
            w0, tri = 0, False                       # full for both blocks
        elif j < 4 * g + 4:
            w0, tri = 128 * (j - 4 * g), True        # diag of block g
        elif j < 28 - 4 * g:
            w0, tri = 512, False                     # solo for block 7-g
        else:
            w0, tri = 512 + 128 * (j - (28 - 4 * g)), True  # diag of 7-g
        out.append((j, w0, tri))
    return out


def _pack_tiles(chunks):
    """Greedy-pack chunk windows into [128,1024] score tiles (one exp each).

    Matmul outputs must not cross the PSUM bank boundary (512 f32 cols), so
    a chunk with w0 < 512 (window wider than 512) sits alone at its natural
    position [w0, 1024).  Narrow (w0 >= 512) chunks are greedy-packed,
    flushing when a chunk would straddle the 512 boundary.  Tiles are
    (chunks, lo, hi): exp runs on [lo, hi); chunk entry (j, w0, tri, loc, n).
    """
    def flush(cur):
        hi = max(loc + n for (_j, _w, _t, loc, n) in cur)
        lo = min(loc for (_j, _w, _t, loc, n) in cur)
        return (cur, lo, hi)

    tiles, cur = [], []
    for (j, w0, tri) in chunks:
        n = 1024 - w0
        if w0 < 512:
            if cur:
                tiles.append(flush(cur))
                cur = []
            tiles.append(([(j, w0, tri, w0, n)], w0, 1024))
            continue
        # narrow chunks: two per tile, one per PSUM bank (loc 0 / 512) so
        # the paired score matmuls never write the same bank concurrently
        cur.append((j, w0, tri, 512 * len(cur), n))
        if len(cur) == 2:
            tiles.append(flush(cur))
            cur = []
    if cur:
        tiles.append(flush(cur))
    return tiles


def _weave(plain, diag, k=2):
    """Interleave diag (PE-light, ACT-heavy) units among plain units."""
    out, pi, di = [], 0, 0
    while pi < len(plain) or di < len(diag):
        for _ in range(k):
            if pi < len(plain):
                out.append(plain[pi]); pi += 1
        if di < len(diag):
            out.append(diag[di]); di += 1
    return out


def _unit_order(g):
    tiles = _pack_tiles(_chunk_windows(g))
    plain = [t for t in tiles if not any(c[2] for c in t[0])]
    diag = [t for t in tiles if any(c[2] for c in t[0])]
    return _weave(plain, diag)


def _segs(w0):
    # matmul free dim is capped at 512 f32 (one PSUM bank), bank-aligned
    return [(w0, 512), (512, 1024)] if w0 < 512 else [(w0, 1024)]


def _build():
    nc = bacc.Bacc(
        "TRN2",
        target_bir_lowering=False,
        debug=False,
        enable_asserts=True,
        num_devices=8,
    )

    xqt_d = nc.declare_dram_parameter("xqt", [P, NCH * NQ], BF, isOutput=False)
    xkt_d = nc.declare_dram_parameter("xkt", [P, NCH * S], BF, isOutput=False)
    xvt_d = nc.declare_dram_parameter("xvt", [P, NCH * S], BF, isOutput=False)
    wq_d = nc.declare_dram_parameter("wq", [P, NCH * E], BF, isOutput=False)
    wk_d = nc.declare_dram_parameter("wk", [P, NCH * E], BF, isOutput=False)
    wv_d = nc.declare_dram_parameter("wv", [P, NCH * E], BF, isOutput=False)
    tri_d = nc.declare_dram_parameter("tri", [P, P], BF, isOutput=False)
    ident_d = nc.declare_dram_parameter("ident", [E, E], BF, isOutput=False)
    zout = nc.declare_dram_parameter("z", [2 * (E + 1), 1024], F32, isOutput=True)

    with tile.TileContext(nc) as tc:
        with (
            tc.tile_pool(name="const", bufs=1) as const,
            tc.tile_pool(name="xt", bufs=1) as xt,
            tc.tile_pool(name="proj", bufs=1) as proj,
            tc.tile_pool(name="work", bufs=3) as work,
            tc.tile_pool(name="epi", bufs=2) as epi,
            tc.tile_pool(name="psA", bufs=2, space="PSUM") as psA,
            tc.tile_pool(name="psZ", bufs=1, space="PSUM") as psZ,
            tc.tile_pool(name="psP", bufs=2, space="PSUM") as psP,
        ):
            wq_sb = const.tile([P, NCH, E], BF, tag="wq")
            wk_sb = const.tile([P, NCH, E], BF, tag="wk")
            wv_sb = const.tile([P, NCH, E], BF, tag="wv")
            tri_sb = const.tile([P, P], BF, tag="tri")
            ident_sb = const.tile([E, E], BF, tag="ident")

            xqt = xt.tile([P, NCH * NQ], BF, tag="xqt")
            xkt = xt.tile([P, NCH * S], BF, tag="xkt")
            xvt = xt.tile([P, NCH * S], BF, tag="xvt")

            kt = proj.tile([P, S], BF, tag="kt")        # K^T, duplicated in
            # partitions 0-63 and 64-127 so score matmuls can run pairwise on
            # split PE tiles (rows 0-63 / 64-127) and stream concurrently.
            qt = proj.tile([P, NQ], BF, tag="qt")       # Q^T, dual copy like kt
            vt = proj.tile([E, S], BF, tag="vt")        # V^T (pre-transpose)
            vp = proj.tile([P, NKCH, E + 1], BF, tag="vp")
            nc.gpsimd.memset(vp[:, :, E : E + 1], 1.0)

            # ---------------- DMA plan: 3 queues, priority order -----------
            SQ, AQ, PQ = nc.sync, nc.scalar, nc.gpsimd

            def wdma(qeng, dst, src):
                qeng.dma_start(
                    out=dst[:, :, :], in_=src.rearrange("p (c e) -> p c e", c=NCH)
                )

            def xpiece(qeng, dst, src, a, b):
                qeng.dma_start(
                    out=dst[:, :].rearrange("p (c s) -> p c s", c=NCH)[:, :, a:b],
                    in_=src.rearrange("p (c s) -> p c s", c=NCH)[:, :, a:b],
                )

            # Front: Q split over sync+scalar (it gates attention start), K/V
            # first pieces on gpsimd; bulk striped over all three queues.
            # Scalar (ACT) carries few issues so its queue drains before exp.
            wdma(AQ, wq_sb, wq_d)
            wdma(SQ, wk_sb, wk_d)
            wdma(PQ, wv_sb, wv_d)
            xpiece(AQ, xqt, xqt_d, 0, 256)
            xpiece(SQ, xqt, xqt_d, 256, 512)
            xpiece(PQ, xkt, xkt_d, 0, 256)
            xpiece(AQ, xqt, xqt_d, 512, 768)
            xpiece(SQ, xqt, xqt_d, 768, 1024)
            xpiece(PQ, xvt, xvt_d, 0, 256)
            AQ.dma_start(out=tri_sb[:, :], in_=tri_d[:, :])
            AQ.dma_start(out=ident_sb[:, :], in_=ident_d[:, :])

            # partition id loads before the bulk queue traffic
            pid = nc.partition_id()

            # PE pstate warmup: the tensor engine ramps to full clock only
            # after ~3us of continuous work.  While the first DMA pieces are
            # in flight the PE is idle anyway — feed it zero matmuls so the
            # real projections start at full speed.
            warm = const.tile([E, 512], BF, tag="warm")
            nc.vector.memset(warm[:, :], 0.0)
            for _ in range(16):
                wps = psP.tile([P, 512], F32, tag="pp")
                nc.tensor.matmul(
                    wps,
                    lhsT=warm[:, 0:P],
                    rhs=warm[:, :],
                    start=True,
                    stop=True,
                )

            for (t, a, b, q) in [
                ("k", 256, 512, SQ), ("v", 256, 512, PQ),
                ("k", 512, 1024, SQ), ("v", 512, 1024, PQ),
                ("k", 1024, 1536, AQ), ("v", 1024, 1536, SQ),
                ("k", 1536, 2048, PQ), ("v", 1536, 2048, SQ),
                ("k", 2048, 2560, AQ), ("v", 2048, 2560, PQ),
                ("q", 1024, 1536, SQ), ("q", 1536, 2048, AQ),
                ("k", 2560, 3072, SQ), ("v", 2560, 3072, PQ),
                ("k", 3072, 3584, SQ), ("v", 3072, 3584, PQ),
                ("k", 3584, 4096, AQ), ("v", 3584, 4096, SQ),
            ]:
                dst, src = {"k": (xkt, xkt_d), "v": (xvt, xvt_d), "q": (xqt, xqt_d)}[t]
                xpiece(q, dst, src, a, b)

            # ---------------- projection emitters (lazy) --------------------
            def emit_kgroup(a, b):
                ps = psP.tile([E, b - a], F32, tag="pp")
                for c in range(NCH):
                    nc.tensor.matmul(
                        ps,
                        lhsT=wk_sb[:, c, :],
                        rhs=xkt[:, c * S + a : c * S + b],
                        start=(c == 0),
                        stop=(c == NCH - 1),
                    )
                nc.vector.tensor_copy(kt[0:E, a:b], ps)
                nc.vector.tensor_copy(kt[E : 2 * E, a:b], ps)

            def emit_qgroup(a, b):
                ps = psP.tile([E, b - a], F32, tag="pp")
                for c in range(NCH):
                    nc.tensor.matmul(
                        ps,
                        lhsT=wq_sb[:, c, :],
                        rhs=xqt[:, c * NQ + a : c * NQ + b],
                        start=(c == 0),
                        stop=(c == NCH - 1),
                    )
                nc.vector.tensor_copy(qt[0:E, a:b], ps)
                nc.vector.tensor_copy(qt[E : 2 * E, a:b], ps)

            def emit_vtgroup(a, b):
                # V^T like K^T: big-N matmuls (32 instrs total instead of 128
                # tiny N=64 ones whose ldweights dominate PE time).
                ps = psP.tile([E, b - a], F32, tag="pp")
                for c in range(NCH):
                    nc.tensor.matmul(
                        ps,
                        lhsT=wv_sb[:, c, :],
                        rhs=xvt[:, c * S + a : c * S + b],
                        start=(c == 0),
                        stop=(c == NCH - 1),
                    )
                nc.vector.tensor_copy(vt[:, a:b], ps)

            state = {"kg": 0, "vg": 0, "vc": 0, "qg": 0}

            def ensure_k(upto_col):
                while state["kg"] < len(KGROUPS) and KGROUPS[state["kg"]][0] < upto_col:
                    a, b = KGROUPS[state["kg"]]
                    emit_kgroup(a, b)
                    state["kg"] += 1

            def ensure_v(upto_chunk):
                upto_chunk = min(upto_chunk, NKCH)
                while (
                    state["vg"] < len(KGROUPS)
                    and KGROUPS[state["vg"]][0] < P * upto_chunk
                ):
                    a, b = KGROUPS[state["vg"]]
                    emit_vtgroup(a, b)
                    state["vg"] += 1
                while state["vc"] < upto_chunk:
                    # transpose four 128-key chunks into one PSUM tile, then
                    # one DVE copy into vp (keeps the psP ring turnover low)
                    i0 = state["vc"]
                    nq = min(4, upto_chunk - i0)
                    tp = psP.tile([P, 4, E], BF, tag="pp")
                    for k in range(nq):
                        nc.tensor.transpose(
                            tp[:, k, :],
                            vt[:, P * (i0 + k) : P * (i0 + k + 1)],
                            ident_sb[:, :],
                        )
                    nc.vector.tensor_copy(vp[:, i0 : i0 + nq, 0:E], tp[:, 0:nq, :])
                    state["vc"] += nq

            def ensure_q(upto_col):
                while state["qg"] < len(QGROUPS) and QGROUPS[state["qg"]][0] < upto_col:
                    a, b = QGROUPS[state["qg"]]
                    emit_qgroup(a, b)
                    state["qg"] += 1

            # ---------------- attention slot emitter ------------------------
            def emit_slot(g, qbase, slot_idx):
                ensure_q(qbase + 1024)
                units = _unit_order(g)
                nch = 32 - 4 * g
                jlast_b0 = 4 * g + 3  # last chunk writing PSUM bank 0
                zp = psZ.tile([E + 1, 1024], F32, tag="zps")

                # execution order of chunks -> per-bank first/last writers
                order = [c for (tc_, lo, hi) in units for c in tc_]
                first_w = {0: None, 1: None}
                last_w = {0: None, 1: None}
                for (j, w0, tri, loc, n) in order:
                    for (a, b) in _segs(w0):
                        bank = a // 512
                        if first_w[bank] is None:
                            first_w[bank] = j
                        last_w[bank] = j
                last_b0_unit = max(
                    ui
                    for ui, (tc_, lo, hi) in enumerate(units)
                    if any(c[1] < 512 for c in tc_)
                )

                def ep_half(bank):
                    zs = epi.tile([E + 1, 512], F32, tag="zsb")
                    nc.vector.tensor_copy(zs, zp[:, 512 * bank : 512 * (bank + 1)])
                    (SQ if bank == 0 else AQ).dma_start(
                        out=zout[
                            (E + 1) * slot_idx : (E + 1) * (slot_idx + 1),
                            512 * bank : 512 * (bank + 1),
                        ],
                        in_=zs,
                    )

                def emit_pv(ui, tile_chunks, pt):
                    for (j, w0, _tri, loc, n) in tile_chunks:
                        for (a, b) in _segs(w0):
                            bank = a // 512
                            nc.tensor.matmul(
                                zp[:, a:b],
                                lhsT=vp[:, j, :],
                                rhs=pt[:, loc + (a - w0) : loc + (b - w0)],
                                start=(first_w[bank] == j),
                                stop=(last_w[bank] == j),
                                skip_group_check=True,
                            )
                    if ui == last_b0_unit:
                        ep_half(0)

                pending = []
                for ui, (tile_chunks, lo, hi) in enumerate(units):
                    jmax = max(c[0] for c in tile_chunks)
                    ensure_k(P * (jmax + 1))
                    ensure_v(jmax + 1)
                    sp = psA.tile([P, 1024], F32, tag="sps")
                    mms = [
                        (j, w0, a, b)
                        for (j, w0, _tri, loc, n) in tile_chunks
                        for (a, b) in _segs(w0)
                    ]
                    locs = {j: loc for (j, _w, _t, loc, _n) in tile_chunks}
                    assert len(mms) <= 2, mms
                    for h, (j, w0, a, b) in enumerate(mms):
                        loc = locs[j]
                        nc.tensor.matmul(
                            sp[:, loc + (a - w0) : loc + (b - w0)],
                            lhsT=kt[h * E : (h + 1) * E, P * j : P * (j + 1)],
                            rhs=qt[h * E : (h + 1) * E, qbase + a : qbase + b],
                            start=True,
                            stop=True,
                            tile_position=(h * E, 0),
                        )
                    pt = work.tile([P, 1024], BF, tag="pt")
                    nc.scalar.activation(
                        out=pt[:, lo:hi],
                        in_=sp[:, lo:hi],
                        func=mybir.ActivationFunctionType.Exp,
                    )
                    for (j, w0, tri, loc, n) in tile_chunks:
                        if tri:
                            nc.gpsimd.tensor_mul(
                                pt[:, loc : loc + P],
                                pt[:, loc : loc + P],
                                tri_sb[:, 0:P],
                            )
                    pending.append((ui, tile_chunks, pt))
                    if len(pending) > 2:
                        emit_pv(*pending.pop(0))
                for args in pending:
                    emit_pv(*args)
                ep_half(1)

            # ---------------- per-parity branches ---------------------------
            # Lazy-projection state must reset per branch: each tc.If body
            # needs its own copy of the post-preamble projections, since a
            # core executes only one branch.
            state_preamble = dict(state)

            def emit_branch(h):
                state.clear()
                state.update(state_preamble)
                gA, gB = PAIR_G[h]
                emit_slot(gA, 0, 0)
                ensure_q(NQ)
                ensure_k(S)
                ensure_v(NKCH)
                emit_slot(gB, 1024, 1)

            with tc.If(pid % 2 == 0):
                emit_branch(0)
            with tc.If(pid % 2 == 1):
                emit_branch(1)

    nc.compile()
    return nc


def _get_nc():
    if "nc" not in _CACHE:
        _CACHE["nc"] = _build()
    return _CACHE["nc"]


def _ensure_ntff_hook():
    """Install antenv.axon_hooks + NTFF profile hook if the image lacks it."""
    import types

    try:
        from antenv import axon_hooks  # noqa: F401

        return
    except ImportError:
        pass
    import antenv
    from concourse import bass_utils as _bu

    mod = types.ModuleType("antenv.axon_hooks")
    _state = {}
    mod.set_axon_ntff_profile_hook = lambda h: _state.__setitem__("h", h)
    mod.get_axon_ntff_profile_hook = lambda: _state.get("h")
    sys.modules["antenv.axon_hooks"] = mod
    antenv.axon_hooks = mod
    sys.path.insert(0, "/root/.axon_site/trn_agent_boot")
    from trn_boot import _ntff_profile_via_ctypes

    mod.set_axon_ntff_profile_hook(
        _ntff_profile_via_ctypes("/opt/axon/libaxon_pjrt.so")
    )
    _bu.upload_artifacts = lambda tmpdir: f"local://{tmpdir}"


def _to_sb_layout(xT):
    """[D, N] (f32) -> [128, NCH*N] bf16 in [p, (c n)] SBUF layout."""
    Dd, N = xT.shape
    return np.ascontiguousarray(
        xT.reshape(NCH, P, N).transpose(1, 0, 2).reshape(P, NCH * N)
    ).astype(BF16)


def kernel(key_inputs, value_inputs, query_inputs, Wq, Wk, Wv):
    global LAST_RESULT
    import os

    key_inputs = np.asarray(key_inputs, dtype=np.float32)
    value_inputs = np.asarray(value_inputs, dtype=np.float32)
    query_inputs = np.asarray(query_inputs, dtype=np.float32)
    wq_b = np.ascontiguousarray(
        (np.asarray(Wq, np.float32) * 0.125).reshape(NCH, P, E).transpose(1, 0, 2)
    ).astype(BF16).reshape(P, NCH * E)
    wk_b = np.ascontiguousarray(
        np.asarray(Wk, np.float32).reshape(NCH, P, E).transpose(1, 0, 2)
    ).astype(BF16).reshape(P, NCH * E)
    wv_b = np.ascontiguousarray(
        np.asarray(Wv, np.float32).reshape(NCH, P, E).transpose(1, 0, 2)
    ).astype(BF16).reshape(P, NCH * E)
    tri_np = (
        (np.arange(P)[:, None] <= np.arange(P)[None, :]).astype(np.float32)
    ).astype(BF16)
    ident_np = np.eye(E, dtype=np.float32).astype(BF16)

    in_maps = []
    for c in range(8):
        b, h = c // 2, c % 2
        order = _block_order(h)
        xq_loc = np.concatenate(
            [query_inputs[b, 512 * g : 512 * (g + 1)] for g in order], axis=0
        )
        in_maps.append(
            {
                "xqt": _to_sb_layout(xq_loc.T),
                "xkt": _to_sb_layout(key_inputs[b].T),
                "xvt": _to_sb_layout(value_inputs[b].T),
                "wq": wq_b,
                "wk": wk_b,
                "wv": wv_b,
                "tri": tri_np,
                "ident": ident_np,
            }
        )

    nc = _get_nc()
    trace = bool(int(os.environ.get("KERNEL_TRACE", "0")))
    if trace:
        _ensure_ntff_hook()
    res = run_bass_kernel_spmd(
        nc,
        in_maps,
        core_ids=list(range(8)),
        trace=trace,
        tmpdir=os.environ.get("KERNEL_TRACE_DIR") or None,
    )
    LAST_RESULT = res

    out = np.empty((B, S, E), dtype=np.float32)
    for c in range(8):
        b, h = c // 2, c % 2
        order = _block_order(h)
        zr = np.asarray(res.results[c]["z"], dtype=np.float32).reshape(2, E + 1, 1024)
        for s in range(2):
            zT = zr[s, 0:E, :] / zr[s, E, :][None, :]
            out[b, 512 * order[2 * s] : 512 * (order[2 * s] + 1)] = zT[:, 0:512].T
            out[b, 512 * order[2 * s + 1] : 512 * (order[2 * s + 1] + 1)] = zT[
                :, 512:1024
            ].T
    return out
